# revision 22
# baseline (speedup 1.0000x reference)
"""Trainium2 Bass kernel for the 3-layer ConvLSTM + dense head model.

Sharding: data-parallel over batch (16/8 = 2 per core) for the ConvLSTM
stack; column-sharded Dense1 (each core computes 128 of 1024 output
columns in bf16, no collective); Dense2/3 finish on host.

Conv matmuls run in float32r (full-rate fp32 mode on TRN2), computed as
tap-accumulated matmuls in PSUM:
  - hidden (SAME) convs read shifted views of a zero-padded SBUF h state
  - input (VALID) convs are fused into the same PSUM accumulation group
  - layer-1's input conv uses host-side im2col (K=125)
  - 64-channel contractions pack 25 taps into 13 matmuls via two
    partition-duplicated copies: shift-1 (in-row pairs) and shift-wp
    (cross-row pairs for the dx=4 column), singleton (4,4) zero-padded
"""
import os
import sys
import types
from contextlib import ExitStack

import ml_dtypes
import numpy as np

import concourse.bacc as bacc
import concourse.bass as bass
import concourse.mybir as mybir
import concourse.tile as tile
from concourse.bass_utils import run_bass_kernel_spmd

F32 = mybir.dt.float32
F32R = mybir.dt.float32r
BF16 = mybir.dt.bfloat16
BFNP = ml_dtypes.bfloat16
AF = mybir.ActivationFunctionType
ALU = mybir.AluOpType

NCORES = 8
BC = 2          # batch per core
T = 6

# layer geometry
L1 = dict(Ho=60, Wo=60, F=128, NCT=4, Wp=64, Cin=128)
L2 = dict(Ho=56, Wo=56, F=64, NCT=2, Wp=60, Cin=128)
L3 = dict(Ho=52, Wo=52, F=64, NCT=2, Wp=56, Cin=64)

LAST_EXEC_NS = []
LAST_RESULTS = []

_CACHE = {}


def _want_trace():
    if os.environ.get("BASS_KERNEL_TRACE") != "1":
        return False
    try:
        _install_ntff_hook()
        return True
    except Exception:
        return False


def _install_ntff_hook():
    if "antenv.axon_hooks" in sys.modules:
        return
    mod = types.ModuleType("antenv.axon_hooks")
    mod._hook = None
    mod.set_axon_ntff_profile_hook = lambda h: setattr(mod, "_hook", h)
    mod.get_axon_ntff_profile_hook = lambda: mod._hook
    sys.modules["antenv.axon_hooks"] = mod
    import antenv
    antenv.axon_hooks = mod
    from trn_agent_boot.trn_boot import _ntff_profile_via_ctypes
    hook = _ntff_profile_via_ctypes("/opt/axon/libaxon_pjrt.so")
    if hook is not None:
        mod.set_axon_ntff_profile_hook(hook)


def _tap_view(t_ap, off, nrow, wp, wo):
    """[128, nrow, wo] strided view at free-dim offset `off`, row stride wp."""
    return t_ap[:, off:off + nrow * wp].rearrange("p (r w) -> p r w", r=nrow)[:, :, :wo]


def _blocks(ho, nrow):
    """Split ho rows into blocks of nrow (last blocks adjusted to keep N>=256)."""
    if ho == 60:                # L1: 6x8 + 2x6 (min N = 6*60=360)
        return [(i * 8, 8) for i in range(6)] + [(48, 6), (54, 6)]
    if ho == 56:                # L2: 7x8 (N = 448)
        return [(i * 8, 8) for i in range(7)]
    if ho == 52:                # L3: 5x9 + 1x7 (N = 468 / 364)
        return [(i * 9, 9) for i in range(5)] + [(45, 7)]
    raise ValueError(ho)


def _build_launch_a(debug=False):
    nc = bacc.Bacc("TRN2", target_bir_lowering=False, debug=False,
                   num_devices=NCORES)

    xim = nc.dram_tensor("xim", [BC * T, 125, 3600], F32R, kind="ExternalInput").ap()
    wx1 = nc.dram_tensor("wx1", [125, 512], F32R, kind="ExternalInput").ap()
    wh1 = nc.dram_tensor("wh1", [128, 25 * 512], F32R, kind="ExternalInput").ap()
    wx2 = nc.dram_tensor("wx2", [128, 25 * 256], F32R, kind="ExternalInput").ap()
    wh2p = nc.dram_tensor("wh2p", [128, 13 * 256], F32R, kind="ExternalInput").ap()
    wx3p = nc.dram_tensor("wx3p", [128, 13 * 256], F32R, kind="ExternalInput").ap()
    wh3p = nc.dram_tensor("wh3p", [128, 13 * 256], F32R, kind="ExternalInput").ap()
    b1v = nc.dram_tensor("b1v", [128, 4], F32, kind="ExternalInput").ap()
    b2v = nc.dram_tensor("b2v", [64, 4], F32, kind="ExternalInput").ap()
    b3v = nc.dram_tensor("b3v", [64, 4], F32, kind="ExternalInput").ap()
    h3o = nc.dram_tensor("h3o", [128, 2704], F32R, kind="ExternalOutput").ap()
    if debug:
        h1dbg = nc.dram_tensor("h1dbg", [BC * T, 128, 3600], F32R, kind="ExternalOutput").ap()
        h2dbg = nc.dram_tensor("h2dbg", [BC * T, 64, 3136], F32R, kind="ExternalOutput").ap()

    with TileCtx(nc) as tc, ExitStack() as top:
        dram = top.enter_context(tc.tile_pool(name="dram", bufs=1, space="DRAM"))
        if debug:
            h1seq, h2seq = h1dbg, h2dbg
        else:
            h1seq = dram.tile([BC * T, 128, 3600], F32R)
            h2seq = dram.tile([BC * T, 64, 3136], F32R)

        # ---------------- phase 1: ConvLSTM(5 -> 128), 60x60 ----------------
        with ExitStack() as ctx:
            wpool = ctx.enter_context(tc.tile_pool(name="w1", bufs=1))
            spool = ctx.enter_context(tc.tile_pool(name="s1", bufs=1))
            ipool = ctx.enter_context(tc.tile_pool(name="i1", bufs=6))
            gpool = ctx.enter_context(tc.tile_pool(name="g1", bufs=2))
            ppool = ctx.enter_context(tc.tile_pool(name="p1", bufs=8, space="PSUM"))

            wx1_t = wpool.tile([125, 512], F32R, tag="wx1")
            wh1_t = wpool.tile([128, 25 * 512], F32R, tag="wh1")
            nc.gpsimd.dma_start(out=wx1_t[:, :], in_=wx1[:, :])
            nc.gpsimd.dma_start(out=wh1_t[:, :], in_=wh1[:, :])
            bsb = wpool.tile([128, 4], F32, tag="b1raw")
            bsig = wpool.tile([128, 4], F32, tag="b1sig")
            nc.sync.dma_start(out=bsb[:, :], in_=b1v[:, :])
            nc.vector.tensor_scalar(bsig[:, :], bsb[:, :], 0.2, 0.5, ALU.mult, ALU.add)

            hpad = [spool.tile([128, 4160], F32R, tag=f"hpad{b}", name=f"hpad1_{b}") for b in range(BC)]
            hcur = [spool.tile([128, 3600], F32R, tag=f"hcur{b}", name=f"hcur1_{b}") for b in range(BC)]
            cst = [spool.tile([128, 3600], F32, tag=f"c1_{b}", name=f"c1_{b}") for b in range(BC)]
            for b in range(BC):
                nc.gpsimd.memset(hpad[b][:, :].bitcast(F32), 0.0)

            wp, wo = L1["Wp"], L1["Wo"]
            for t in range(T):
                for b in range(BC):
                    cts = (0, 2, 3) if t == 0 else (0, 1, 2, 3)
                    for y0, nrow in _blocks(60, 8):
                        n = nrow * wo
                        xb = ipool.tile([125, 480], F32R, tag="xim")
                        nc.sync.dma_start(out=xb[:, :n],
                                          in_=xim[b * T + t, :, y0 * 60:y0 * 60 + n])
                        ps = {}
                        for ct in cts:
                            acc = ppool.tile([128, 480], F32, tag="ps")
                            ps[ct] = acc
                            nc.tensor.matmul(
                                acc[:, :n], wx1_t[:, ct * 128:(ct + 1) * 128],
                                xb[:, :n],
                                start=True, stop=(t == 0))
                            if t > 0:
                                for tap in range(25):
                                    dy, dx = divmod(tap, 5)
                                    rhs = _tap_view(hpad[b], (y0 + dy) * wp + dx, nrow, wp, wo)
                                    nc.tensor.matmul(
                                        acc[:, :n],
                                        wh1_t[:, tap * 512 + ct * 128:tap * 512 + (ct + 1) * 128],
                                        rhs, start=False, stop=(tap == 24))
                        sl = slice(y0 * wo, y0 * wo + n)
                        g = {}
                        for ct in cts:
                            gt = gpool.tile([128, 480], F32, tag=f"g{ct}")
                            g[ct] = gt
                            if ct == 2:
                                nc.scalar.activation(gt[:, :n], ps[ct][:, :n], AF.Tanh,
                                                     bias=bsb[:, 2:3])
                            else:
                                nc.scalar.activation(gt[:, :n], ps[ct][:, :n], AF.Identity,
                                                     bias=bsig[:, ct:ct + 1], scale=0.2)
                                nc.vector.tensor_scalar(gt[:, :n], gt[:, :n], 0.0, 1.0,
                                                        ALU.max, ALU.min)
                        if t == 0:
                            nc.vector.tensor_mul(cst[b][:, sl], g[0][:, :n], g[2][:, :n])
                        else:
                            t1 = gpool.tile([128, 480], F32, tag="t1")
                            t2 = gpool.tile([128, 480], F32, tag="t2")
                            nc.vector.tensor_mul(t1[:, :n], g[1][:, :n], cst[b][:, sl])
                            nc.vector.tensor_mul(t2[:, :n], g[0][:, :n], g[2][:, :n])
                            nc.vector.tensor_add(cst[b][:, sl], t1[:, :n], t2[:, :n])
                        tc_t = gpool.tile([128, 480], F32, tag="tct")
                        nc.scalar.activation(tc_t[:, :n], cst[b][:, sl], AF.Tanh)
                        nc.vector.tensor_mul(hcur[b][:, sl], g[3][:, :n], tc_t[:, :n])
                    # end blocks: update padded state + spill sequence
                    dst = _tap_view(hpad[b], 2 * wp + 2, wo, wp, wo)
                    src = hcur[b][:, :].rearrange("p (r w) -> p r w", r=wo)
                    nc.vector.tensor_copy(dst, src)
                    nc.sync.dma_start(out=h1seq[b * T + t, :, :], in_=hcur[b][:, :])

        # ---------------- phase 2: ConvLSTM(128 -> 64), 56x56 ----------------
        with ExitStack() as ctx:
            wpool = ctx.enter_context(tc.tile_pool(name="w2", bufs=1))
            spool = ctx.enter_context(tc.tile_pool(name="s2", bufs=1))
            ipool = ctx.enter_context(tc.tile_pool(name="i2", bufs=2))
            gpool = ctx.enter_context(tc.tile_pool(name="g2", bufs=2))
            ppool = ctx.enter_context(tc.tile_pool(name="p2", bufs=8, space="PSUM"))

            wx2_t = wpool.tile([128, 25 * 256], F32R, tag="wx2")
            wh2_t = wpool.tile([128, 13 * 256], F32R, tag="wh2")
            nc.gpsimd.dma_start(out=wx2_t[:, :], in_=wx2[:, :])
            nc.gpsimd.dma_start(out=wh2_t[:, :], in_=wh2p[:, :])
            bsb = wpool.tile([64, 4], F32, tag="b2raw")
            bsig = wpool.tile([64, 4], F32, tag="b2sig")
            nc.sync.dma_start(out=bsb[:, :], in_=b2v[:, :])
            nc.vector.tensor_scalar(bsig[:, :], bsb[:, :], 0.2, 0.5, ALU.mult, ALU.add)

            hpad = [spool.tile([128, 3664], F32R, tag=f"hpad{b}", name=f"hpad2_{b}") for b in range(BC)]
            hpadB = [spool.tile([128, 3664], F32R, tag=f"hpadB{b}", name=f"hpad2B_{b}") for b in range(BC)]
            hcur = [spool.tile([64, 3136], F32R, tag=f"hcur{b}", name=f"hcur2_{b}") for b in range(BC)]
            cst = [spool.tile([64, 3136], F32, tag=f"c2_{b}", name=f"c2_{b}") for b in range(BC)]
            for b in range(BC):
                nc.gpsimd.memset(hpad[b][:, :].bitcast(F32), 0.0)
                nc.gpsimd.memset(hpadB[b][:, :].bitcast(F32), 0.0)

            wp, wo = L2["Wp"], L2["Wo"]
            for t in range(T):
                for b in range(BC):
                    img = ipool.tile([128, 3640], F32R, tag="h1in")
                    nc.sync.dma_start(out=img[:, :3600], in_=h1seq[b * T + t, :, :])
                    for y0, nrow in _blocks(56, 8):
                        n = nrow * wo
                        ps = []
                        for ct in range(2):
                            acc = ppool.tile([128, 448], F32, tag="ps")
                            ps.append(acc)
                            first = True
                            for tap in range(25):
                                dy, dx = divmod(tap, 5)
                                rhs = _tap_view(img, (y0 + dy) * 60 + dx, nrow, 60, wo)
                                nc.tensor.matmul(
                                    acc[:, :n],
                                    wx2_t[:, tap * 256 + ct * 128:tap * 256 + (ct + 1) * 128],
                                    rhs, start=first,
                                    stop=(t == 0 and tap == 24))
                                first = False
                            if t > 0:
                                for e in range(13):
                                    if e < 10:
                                        dy, k = divmod(e, 2)
                                        src, off = hpad[b], (y0 + dy) * wp + 2 * k
                                    elif e < 12:
                                        dy = 2 * (e - 10)
                                        src, off = hpadB[b], (y0 + dy) * wp + 4
                                    else:
                                        src, off = hpad[b], (y0 + 4) * wp + 4
                                    rhs = _tap_view(src, off, nrow, wp, wo)
                                    nc.tensor.matmul(
                                        acc[:, :n],
                                        wh2_t[:, e * 256 + ct * 128:e * 256 + (ct + 1) * 128],
                                        rhs, start=False, stop=(e == 12))
                        sl = slice(y0 * wo, y0 * wo + n)
                        # gates: ps[0]=[i;f], ps[1]=[c;o]
                        si = gpool.tile([64, 448], F32, tag="si")
                        nc.scalar.activation(si[:, :n], ps[0][0:64, :n], AF.Identity,
                                             bias=bsig[:, 0:1], scale=0.2)
                        nc.vector.tensor_scalar(si[:, :n], si[:, :n], 0.0, 1.0,
                                                ALU.max, ALU.min)
                        gt = gpool.tile([64, 448], F32, tag="gt")
                        nc.scalar.activation(gt[:, :n], ps[1][0:64, :n], AF.Tanh,
                                             bias=bsb[:, 2:3])
                        so = gpool.tile([64, 448], F32, tag="so")
                        nc.scalar.activation(so[:, :n], ps[1][64:128, :n], AF.Identity,
                                             bias=bsig[:, 3:4], scale=0.2)
                        nc.vector.tensor_scalar(so[:, :n], so[:, :n], 0.0, 1.0,
                                                ALU.max, ALU.min)
                        if t == 0:
                            nc.vector.tensor_mul(cst[b][:, sl], si[:, :n], gt[:, :n])
                        else:
                            sf = gpool.tile([64, 448], F32, tag="sf")
                            nc.scalar.activation(sf[:, :n], ps[0][64:128, :n], AF.Identity,
                                                 bias=bsig[:, 1:2], scale=0.2)
                            nc.vector.tensor_scalar(sf[:, :n], sf[:, :n], 0.0, 1.0,
                                                    ALU.max, ALU.min)
                            t1 = gpool.tile([64, 448], F32, tag="t1")
                            t2 = gpool.tile([64, 448], F32, tag="t2")
                            nc.vector.tensor_mul(t1[:, :n], sf[:, :n], cst[b][:, sl])
                            nc.vector.tensor_mul(t2[:, :n], si[:, :n], gt[:, :n])
                            nc.vector.tensor_add(cst[b][:, sl], t1[:, :n], t2[:, :n])
                        tc_t = gpool.tile([64, 448], F32, tag="tct")
                        nc.scalar.activation(tc_t[:, :n], cst[b][:, sl], AF.Tanh)
                        nc.vector.tensor_mul(hcur[b][:, sl], so[:, :n], tc_t[:, :n])
                    # end blocks: dup states (A: plain/x+1-shift, B: plain/row-shift)
                    src = hcur[b][:, :].rearrange("p (r w) -> p r w", r=wo)
                    dst0 = hpad[b][0:64, 2 * wp + 2:2 * wp + 2 + wo * wp] \
                        .rearrange("p (r w) -> p r w", r=wo)[:, :, :wo]
                    nc.vector.tensor_copy(dst0, src)
                    dst1 = hpad[b][64:128, 2 * wp + 1:2 * wp + 1 + wo * wp] \
                        .rearrange("p (r w) -> p r w", r=wo)[:, :, :wo]
                    nc.vector.tensor_copy(dst1, src)
                    dstB0 = hpadB[b][0:64, 2 * wp + 2:2 * wp + 2 + wo * wp] \
                        .rearrange("p (r w) -> p r w", r=wo)[:, :, :wo]
                    nc.vector.tensor_copy(dstB0, src)
                    dstB1 = hpadB[b][64:128, wp + 2:wp + 2 + wo * wp] \
                        .rearrange("p (r w) -> p r w", r=wo)[:, :, :wo]
                    nc.vector.tensor_copy(dstB1, src)
                    nc.sync.dma_start(out=h2seq[b * T + t, :, :], in_=hcur[b][:, :])

        # ---------------- phase 3: ConvLSTM(64 -> 64), 52x52 ----------------
        with ExitStack() as ctx:
            wpool = ctx.enter_context(tc.tile_pool(name="w3", bufs=1))
            spool = ctx.enter_context(tc.tile_pool(name="s3", bufs=1))
            ipool = ctx.enter_context(tc.tile_pool(name="i3", bufs=2))
            gpool = ctx.enter_context(tc.tile_pool(name="g3", bufs=2))
            ppool = ctx.enter_context(tc.tile_pool(name="p3", bufs=8, space="PSUM"))

            wx3_t = wpool.tile([128, 13 * 256], F32R, tag="wx3")
            wh3_t = wpool.tile([128, 13 * 256], F32R, tag="wh3")
            nc.gpsimd.dma_start(out=wx3_t[:, :], in_=wx3p[:, :])
            nc.gpsimd.dma_start(out=wh3_t[:, :], in_=wh3p[:, :])
            bsb = wpool.tile([64, 4], F32, tag="b3raw")
            bsig = wpool.tile([64, 4], F32, tag="b3sig")
            nc.sync.dma_start(out=bsb[:, :], in_=b3v[:, :])
            nc.vector.tensor_scalar(bsig[:, :], bsb[:, :], 0.2, 0.5, ALU.mult, ALU.add)

            hpad = [spool.tile([128, 3300], F32R, tag=f"hpad{b}", name=f"hpad3_{b}") for b in range(BC)]
            hpadB = [spool.tile([128, 3300], F32R, tag=f"hpadB{b}", name=f"hpad3B_{b}") for b in range(BC)]
            hcur = [spool.tile([64, 2704], F32R, tag=f"hcur{b}", name=f"hcur3_{b}") for b in range(BC)]
            cst = [spool.tile([64, 2704], F32, tag=f"c3_{b}", name=f"c3_{b}") for b in range(BC)]
            for b in range(BC):
                nc.gpsimd.memset(hpad[b][:, :].bitcast(F32), 0.0)
                nc.gpsimd.memset(hpadB[b][:, :].bitcast(F32), 0.0)

            wp, wo = L3["Wp"], L3["Wo"]
            for t in range(T):
                for b in range(BC):
                    # dup inputs [128, 3196]: img rows64 = x+1 shift, imgB rows64 =
                    # one-image-row shift; all four halves DMAed from DRAM.
                    img = ipool.tile([128, 3196], F32R, tag="h2in")
                    imgB = ipool.tile([128, 3196], F32R, tag="h2inB")
                    nc.sync.dma_start(out=img[0:64, :3136], in_=h2seq[b * T + t, :, :])
                    nc.sync.dma_start(out=img[64:128, :3135], in_=h2seq[b * T + t, :, 1:3136])
                    nc.sync.dma_start(out=imgB[0:64, :3136], in_=h2seq[b * T + t, :, :])
                    nc.sync.dma_start(out=imgB[64:128, :3080], in_=h2seq[b * T + t, :, 56:3136])
                    for y0, nrow in _blocks(52, 9):
                        n = nrow * wo
                        ps = []
                        for ct in range(2):
                            acc = ppool.tile([128, 468], F32, tag="ps")
                            ps.append(acc)
                            first = True
                            for e in range(13):
                                if e < 10:
                                    dy, k = divmod(e, 2)
                                    src, off = img, (y0 + dy) * 56 + 2 * k
                                elif e < 12:
                                    dy = 2 * (e - 10)
                                    src, off = imgB, (y0 + dy) * 56 + 4
                                else:
                                    src, off = img, (y0 + 4) * 56 + 4
                                stop13 = (t == 0 and e == 12)
                                if e == 12:
                                    # singleton tap (4,4): K=64 (upper half unwritten)
                                    rhs = src[0:64, off:off + nrow * 56] \
                                        .rearrange("p (r w) -> p r w", r=nrow)[:, :, :wo]
                                    nc.tensor.matmul(
                                        acc[:, :n],
                                        wx3_t[0:64, e * 256 + ct * 128:e * 256 + (ct + 1) * 128],
                                        rhs, start=first, stop=stop13)
                                else:
                                    rhs = _tap_view(src, off, nrow, 56, wo)
                                    nc.tensor.matmul(
                                        acc[:, :n],
                                        wx3_t[:, e * 256 + ct * 128:e * 256 + (ct + 1) * 128],
                                        rhs, start=first, stop=stop13)
                                first = False
                            if t > 0:
                                for e in range(13):
                                    if e < 10:
                                        dy, k = divmod(e, 2)
                                        src, off = hpad[b], (y0 + dy) * wp + 2 * k
                                    elif e < 12:
                                        dy = 2 * (e - 10)
                                        src, off = hpadB[b], (y0 + dy) * wp + 4
                                    else:
                                        src, off = hpad[b], (y0 + 4) * wp + 4
                                    rhs = _tap_view(src, off, nrow, wp, wo)
                                    nc.tensor.matmul(
                                        acc[:, :n],
                                        wh3_t[:, e * 256 + ct * 128:e * 256 + (ct + 1) * 128],
                                        rhs, start=False, stop=(e == 12))
                        sl = slice(y0 * wo, y0 * wo + n)
                        si = gpool.tile([64, 468], F32, tag="si")
                        nc.scalar.activation(si[:, :n], ps[0][0:64, :n], AF.Identity,
                                             bias=bsig[:, 0:1], scale=0.2)
                        nc.vector.tensor_scalar(si[:, :n], si[:, :n], 0.0, 1.0,
                                                ALU.max, ALU.min)
                        gt = gpool.tile([64, 468], F32, tag="gt")
                        nc.scalar.activation(gt[:, :n], ps[1][0:64, :n], AF.Tanh,
                                             bias=bsb[:, 2:3])
                        so = gpool.tile([64, 468], F32, tag="so")
                        nc.scalar.activation(so[:, :n], ps[1][64:128, :n], AF.Identity,
                                             bias=bsig[:, 3:4], scale=0.2)
                        nc.vector.tensor_scalar(so[:, :n], so[:, :n], 0.0, 1.0,
                                                ALU.max, ALU.min)
                        if t == 0:
                            nc.vector.tensor_mul(cst[b][:, sl], si[:, :n], gt[:, :n])
                        else:
                            sf = gpool.tile([64, 468], F32, tag="sf")
                            nc.scalar.activation(sf[:, :n], ps[0][64:128, :n], AF.Identity,
                                                 bias=bsig[:, 1:2], scale=0.2)
                            nc.vector.tensor_scalar(sf[:, :n], sf[:, :n], 0.0, 1.0,
                                                    ALU.max, ALU.min)
                            t1 = gpool.tile([64, 468], F32, tag="t1")
                            t2 = gpool.tile([64, 468], F32, tag="t2")
                            nc.vector.tensor_mul(t1[:, :n], sf[:, :n], cst[b][:, sl])
                            nc.vector.tensor_mul(t2[:, :n], si[:, :n], gt[:, :n])
                            nc.vector.tensor_add(cst[b][:, sl], t1[:, :n], t2[:, :n])
                        tc_t = gpool.tile([64, 468], F32, tag="tct")
                        nc.scalar.activation(tc_t[:, :n], cst[b][:, sl], AF.Tanh)
                        nc.vector.tensor_mul(hcur[b][:, sl], so[:, :n], tc_t[:, :n])
                    if t < T - 1:
                        src = hcur[b][:, :].rearrange("p (r w) -> p r w", r=wo)
                        dst0 = hpad[b][0:64, 2 * wp + 2:2 * wp + 2 + wo * wp] \
                            .rearrange("p (r w) -> p r w", r=wo)[:, :, :wo]
                        nc.vector.tensor_copy(dst0, src)
                        dst1 = hpad[b][64:128, 2 * wp + 1:2 * wp + 1 + wo * wp] \
                            .rearrange("p (r w) -> p r w", r=wo)[:, :, :wo]
                        nc.vector.tensor_copy(dst1, src)
                        dstB0 = hpadB[b][0:64, 2 * wp + 2:2 * wp + 2 + wo * wp] \
                            .rearrange("p (r w) -> p r w", r=wo)[:, :, :wo]
                        nc.vector.tensor_copy(dstB0, src)
                        dstB1 = hpadB[b][64:128, wp + 2:wp + 2 + wo * wp] \
                            .rearrange("p (r w) -> p r w", r=wo)[:, :, :wo]
                        nc.vector.tensor_copy(dstB1, src)
                    else:
                        nc.sync.dma_start(out=h3o[b * 64:(b + 1) * 64, :], in_=hcur[b][:, :])

    nc.compile()
    return nc


def TileCtx(nc):
    return tile.TileContext(nc, pool_alloc_mode="queue")


def _build_launch_b():
    """Column-sharded dense1 in bf16: core j computes a1[:, j*128:(j+1)*128].
    No collective; bias/relu/dense2/dense3 finish on host."""
    D = 173056
    KT = D // 128              # 1352 k-tiles
    CH = 52                    # k-tiles per weight DMA chunk
    NB = 8                     # interleaved PSUM accumulation banks
    nc = bacc.Bacc("TRN2", target_bir_lowering=False, debug=False,
                   num_devices=NCORES)
    ztk = nc.dram_tensor("ztk", [128, KT * 16], BF16, kind="ExternalInput").ap()
    wd1c = nc.dram_tensor("wd1c", [128, KT * 128], BF16, kind="ExternalInput").ap()
    out = nc.dram_tensor("out", [16, 128], F32, kind="ExternalOutput").ap()

    with TileCtx(nc) as tc, ExitStack() as ctx:
        cpool = ctx.enter_context(tc.tile_pool(name="cst", bufs=1))
        wpool = ctx.enter_context(tc.tile_pool(name="wd1", bufs=4))
        ppool = ctx.enter_context(tc.tile_pool(name="ps", bufs=1, space="PSUM"))

        zt = cpool.tile([128, KT * 16], BF16, tag="zt")
        nc.gpsimd.dma_start(out=zt[:, :], in_=ztk[:, :])
        accs = [ppool.tile([16, 128], F32, tag=f"a{b}", name=f"acc{b}")
                for b in range(NB)]

        nchunk = (KT + CH - 1) // CH
        for c in range(nchunk):
            c0 = c * CH
            cn = min(CH, KT - c0)
            w_t = wpool.tile([128, CH * 128], BF16, tag="w", name=f"w{c0}")
            nc.sync.dma_start(out=w_t[:, :cn * 128],
                              in_=wd1c[:, c0 * 128:(c0 + cn) * 128])
            for i in range(cn):
                kt = c0 + i
                nc.tensor.matmul(accs[kt % NB][:, :], zt[:, kt * 16:(kt + 1) * 16],
                                 w_t[:, i * 128:(i + 1) * 128],
                                 start=(kt < NB), stop=(kt >= KT - NB))
        sums = [cpool.tile([16, 128], F32, tag=f"s{i}", name=f"sum{i}")
                for i in range(NB)]
        nc.vector.tensor_copy(sums[0][:, :], accs[0][:, :])
        for i in range(1, NB):
            nc.vector.tensor_add(sums[i][:, :], sums[i - 1][:, :], accs[i][:, :])
        nc.sync.dma_start(out=out[:, :], in_=sums[NB - 1][:, :])

    nc.compile()
    return nc


def _pack13(w):
    """(5,5,64,256) -> [128, 13*256].
    e<10: in-row pairs (dy,2k)+(dy,2k+1) via the shift-1 dup (k=divmod).
    e=10,11: cross-row pairs (dy,4)+(dy+1,4) via the shift-wp dup (dy=0,2).
    e=12: singleton (4,4), upper half zero."""
    out = np.zeros((128, 13, 256), np.float32)
    for e in range(10):
        dy, k = divmod(e, 2)
        out[0:64, e] = w[dy, 2 * k]
        out[64:128, e] = w[dy, 2 * k + 1]
    for i, dy in enumerate((0, 2)):
        out[0:64, 10 + i] = w[dy, 4]
        out[64:128, 10 + i] = w[dy + 1, 4]
    out[0:64, 12] = w[4, 4]
    return np.ascontiguousarray(out.reshape(128, 13 * 256))


def _host_prep_a(x, Wx1, Wh1, b1, Wx2, Wh2, b2, Wx3, Wh3, b3):
    xw = np.lib.stride_tricks.sliding_window_view(x, (5, 5), axis=(2, 3))
    # [b,t,y,x,c,dy,dx] -> [b,t,(dy,dx,c),(y,x)]
    xim = np.ascontiguousarray(
        xw.transpose(0, 1, 5, 6, 4, 2, 3).reshape(16, 6, 125, 3600), np.float32)
    shared = dict(
        wx1=np.ascontiguousarray(Wx1.reshape(125, 512), np.float32),
        wh1=np.ascontiguousarray(
            Wh1.reshape(25, 128, 512).transpose(1, 0, 2).reshape(128, 25 * 512)),
        wx2=np.ascontiguousarray(
            Wx2.reshape(25, 128, 256).transpose(1, 0, 2).reshape(128, 25 * 256)),
        wh2p=_pack13(Wh2.reshape(5, 5, 64, 256)),
        wx3p=_pack13(Wx3.reshape(5, 5, 64, 256)),
        wh3p=_pack13(Wh3.reshape(5, 5, 64, 256)),
        b1v=np.ascontiguousarray(b1.reshape(4, 128).T, np.float32),
        b2v=np.ascontiguousarray(b2.reshape(4, 64).T, np.float32),
        b3v=np.ascontiguousarray(b3.reshape(4, 64).T, np.float32),
    )
    in_maps = []
    for j in range(NCORES):
        m = dict(shared)
        m["xim"] = np.ascontiguousarray(
            xim[2 * j:2 * j + 2].reshape(12, 125, 3600))
        in_maps.append(m)
    return in_maps


def _run(nc, in_maps, trace):
    res = run_bass_kernel_spmd(nc, in_maps, core_ids=list(range(NCORES)),
                               trace=trace)
    if res.exec_time_ns is not None:
        LAST_EXEC_NS.append(res.exec_time_ns)
    LAST_RESULTS.append(res)
    return res


def kernel(x, Wx1, Wh1, b1, Wx2, Wh2, b2, Wx3, Wh3, b3,
           Wd1, bd1, Wd2, bd2, Wd3, bd3):
    trace = _want_trace()
    LAST_EXEC_NS.clear()
    LAST_RESULTS.clear()
    x = np.asarray(x, np.float32)

    if "a" not in _CACHE:
        _CACHE["a"] = _build_launch_a()
    in_a = _host_prep_a(x, np.asarray(Wx1), np.asarray(Wh1), np.asarray(b1),
                        np.asarray(Wx2), np.asarray(Wh2), np.asarray(b2),
                        np.asarray(Wx3), np.asarray(Wh3), np.asarray(b3))
    res_a = _run(_CACHE["a"], in_a, trace)

    h3 = np.stack([res_a.results[j]["h3o"][(b % 2) * 64:(b % 2) * 64 + 64]
                   for b, j in [(b, b // 2) for b in range(16)]])  # [16,64,2704]
    zt = np.ascontiguousarray(h3.transpose(2, 1, 0).reshape(173056, 16), np.float32)

    if "b" not in _CACHE:
        _CACHE["b"] = _build_launch_b()
    D = 173056
    KT = D // 128
    w_b = np.asarray(Wd1, np.float32).astype(BFNP)
    ztk = np.ascontiguousarray(
        zt.astype(BFNP).reshape(KT, 128, 16).transpose(1, 0, 2).reshape(128, KT * 16))
    in_b = []
    for j in range(NCORES):
        shard = w_b[:, j * 128:(j + 1) * 128]
        in_b.append(dict(ztk=ztk, wd1c=np.ascontiguousarray(
            shard.reshape(KT, 128, 128).transpose(1, 0, 2).reshape(128, KT * 128))))
    res_b = _run(_CACHE["b"], in_b, trace)

    a1 = np.concatenate([res_b.results[j]["out"] for j in range(NCORES)], axis=1)
    a1 = np.maximum(a1 + np.asarray(bd1, np.float32), 0.0)
    a2 = np.maximum(a1 @ np.asarray(Wd2, np.float32) + np.asarray(bd2, np.float32), 0.0)
    out = a2 @ np.asarray(Wd3, np.float32) + np.asarray(bd3, np.float32)
    return np.ascontiguousarray(out, np.float32)



# revision 23
# speedup vs baseline: 1.0093x; 1.0093x over previous
"""Trainium2 Bass kernel for the 3-layer ConvLSTM + dense head model.

Sharding: data-parallel over batch (16/8 = 2 per core) for the ConvLSTM
stack; column-sharded Dense1 (each core computes 128 of 1024 output
columns in bf16, no collective); Dense2/3 finish on host.

Conv matmuls run in float32r (full-rate fp32 mode on TRN2), computed as
tap-accumulated matmuls in PSUM:
  - hidden (SAME) convs read shifted views of a zero-padded SBUF h state
  - input (VALID) convs are fused into the same PSUM accumulation group
  - layer-1's input conv uses host-side im2col (K=125)
  - 64-channel contractions pack 25 taps into 13 matmuls via two
    partition-duplicated copies: shift-1 (in-row pairs) and shift-wp
    (cross-row pairs for the dx=4 column), singleton (4,4) zero-padded
"""
import os
import sys
import types
from contextlib import ExitStack

import ml_dtypes
import numpy as np

import concourse.bacc as bacc
import concourse.bass as bass
import concourse.mybir as mybir
import concourse.tile as tile
from concourse.bass_utils import run_bass_kernel_spmd

F32 = mybir.dt.float32
F32R = mybir.dt.float32r
BF16 = mybir.dt.bfloat16
BFNP = ml_dtypes.bfloat16
AF = mybir.ActivationFunctionType
ALU = mybir.AluOpType

NCORES = 8
BC = 2          # batch per core
T = 6

# layer geometry
L1 = dict(Ho=60, Wo=60, F=128, NCT=4, Wp=64, Cin=128)
L2 = dict(Ho=56, Wo=56, F=64, NCT=2, Wp=60, Cin=128)
L3 = dict(Ho=52, Wo=52, F=64, NCT=2, Wp=56, Cin=64)

LAST_EXEC_NS = []
LAST_RESULTS = []

_CACHE = {}


def _want_trace():
    if os.environ.get("BASS_KERNEL_TRACE") != "1":
        return False
    try:
        _install_ntff_hook()
        return True
    except Exception:
        return False


def _install_ntff_hook():
    if "antenv.axon_hooks" in sys.modules:
        return
    mod = types.ModuleType("antenv.axon_hooks")
    mod._hook = None
    mod.set_axon_ntff_profile_hook = lambda h: setattr(mod, "_hook", h)
    mod.get_axon_ntff_profile_hook = lambda: mod._hook
    sys.modules["antenv.axon_hooks"] = mod
    import antenv
    antenv.axon_hooks = mod
    from trn_agent_boot.trn_boot import _ntff_profile_via_ctypes
    hook = _ntff_profile_via_ctypes("/opt/axon/libaxon_pjrt.so")
    if hook is not None:
        mod.set_axon_ntff_profile_hook(hook)


def _tap_view(t_ap, off, nrow, wp, wo):
    """[128, nrow, wo] strided view at free-dim offset `off`, row stride wp."""
    return t_ap[:, off:off + nrow * wp].rearrange("p (r w) -> p r w", r=nrow)[:, :, :wo]


def _blocks(ho, nrow):
    """Split ho rows into blocks of nrow (last blocks adjusted to keep N>=256)."""
    if ho == 60:                # L1: 6x8 + 2x6 (min N = 6*60=360)
        return [(i * 8, 8) for i in range(6)] + [(48, 6), (54, 6)]
    if ho == 56:                # L2: 7x8 (N = 448)
        return [(i * 8, 8) for i in range(7)]
    if ho == 52:                # L3: 5x9 + 1x7 (N = 468 / 364)
        return [(i * 9, 9) for i in range(5)] + [(45, 7)]
    raise ValueError(ho)


def _build_launch_a(debug=False):
    nc = bacc.Bacc("TRN2", target_bir_lowering=False, debug=False,
                   num_devices=NCORES)

    xim = nc.dram_tensor("xim", [BC * T, 125, 3600], F32R, kind="ExternalInput").ap()
    wx1 = nc.dram_tensor("wx1", [125, 512], F32R, kind="ExternalInput").ap()
    wh1 = nc.dram_tensor("wh1", [128, 25 * 512], F32R, kind="ExternalInput").ap()
    wx2 = nc.dram_tensor("wx2", [128, 25 * 256], F32R, kind="ExternalInput").ap()
    wh2p = nc.dram_tensor("wh2p", [128, 13 * 256], F32R, kind="ExternalInput").ap()
    wx3p = nc.dram_tensor("wx3p", [128, 13 * 256], F32R, kind="ExternalInput").ap()
    wh3p = nc.dram_tensor("wh3p", [128, 13 * 256], F32R, kind="ExternalInput").ap()
    b1v = nc.dram_tensor("b1v", [128, 4], F32, kind="ExternalInput").ap()
    b2v = nc.dram_tensor("b2v", [64, 4], F32, kind="ExternalInput").ap()
    b3v = nc.dram_tensor("b3v", [64, 4], F32, kind="ExternalInput").ap()
    h3o = nc.dram_tensor("h3o", [128, 2704], F32R, kind="ExternalOutput").ap()
    if debug:
        h1dbg = nc.dram_tensor("h1dbg", [BC * T, 128, 3600], F32R, kind="ExternalOutput").ap()
        h2dbg = nc.dram_tensor("h2dbg", [BC * T, 64, 3136], F32R, kind="ExternalOutput").ap()

    with TileCtx(nc) as tc, ExitStack() as top:
        dram = top.enter_context(tc.tile_pool(name="dram", bufs=1, space="DRAM"))
        if debug:
            h1seq, h2seq = h1dbg, h2dbg
        else:
            h1seq = dram.tile([BC * T, 128, 3600], F32R)
            h2seq = dram.tile([BC * T, 64, 3136], F32R)

        # ---------------- phase 1: ConvLSTM(5 -> 128), 60x60 ----------------
        with ExitStack() as ctx:
            wpool = ctx.enter_context(tc.tile_pool(name="w1", bufs=1))
            spool = ctx.enter_context(tc.tile_pool(name="s1", bufs=1))
            ipool = ctx.enter_context(tc.tile_pool(name="i1", bufs=6))
            gpool = ctx.enter_context(tc.tile_pool(name="g1", bufs=2))
            ppool = ctx.enter_context(tc.tile_pool(name="p1", bufs=8, space="PSUM"))

            wx1_t = wpool.tile([125, 512], F32R, tag="wx1")
            wh1_t = wpool.tile([128, 25 * 512], F32R, tag="wh1")
            nc.gpsimd.dma_start(out=wx1_t[:, :], in_=wx1[:, :])
            nc.gpsimd.dma_start(out=wh1_t[:, :], in_=wh1[:, :])
            bsb = wpool.tile([128, 4], F32, tag="b1raw")
            bsig = wpool.tile([128, 4], F32, tag="b1sig")
            nc.sync.dma_start(out=bsb[:, :], in_=b1v[:, :])
            nc.vector.tensor_scalar(bsig[:, :], bsb[:, :], 0.2, 0.5, ALU.mult, ALU.add)

            hpad = [spool.tile([128, 4160], F32R, tag=f"hpad{b}", name=f"hpad1_{b}") for b in range(BC)]
            hcur = [spool.tile([128, 3600], F32R, tag=f"hcur{b}", name=f"hcur1_{b}") for b in range(BC)]
            cst = [spool.tile([128, 3600], F32, tag=f"c1_{b}", name=f"c1_{b}") for b in range(BC)]
            for b in range(BC):
                nc.gpsimd.memset(hpad[b][:, :].bitcast(F32), 0.0)

            wp, wo = L1["Wp"], L1["Wo"]
            for t in range(T):
                for b in range(BC):
                    cts = (0, 2, 3) if t == 0 else (0, 1, 2, 3)
                    for y0, nrow in _blocks(60, 8):
                        n = nrow * wo
                        xb = ipool.tile([125, 480], F32R, tag="xim")
                        nc.sync.dma_start(out=xb[:, :n],
                                          in_=xim[b * T + t, :, y0 * 60:y0 * 60 + n])
                        ps = {}
                        for ct in cts:
                            acc = ppool.tile([128, 480], F32, tag="ps")
                            ps[ct] = acc
                            nc.tensor.matmul(
                                acc[:, :n], wx1_t[:, ct * 128:(ct + 1) * 128],
                                xb[:, :n],
                                start=True, stop=(t == 0))
                            if t > 0:
                                for tap in range(25):
                                    dy, dx = divmod(tap, 5)
                                    rhs = _tap_view(hpad[b], (y0 + dy) * wp + dx, nrow, wp, wo)
                                    nc.tensor.matmul(
                                        acc[:, :n],
                                        wh1_t[:, tap * 512 + ct * 128:tap * 512 + (ct + 1) * 128],
                                        rhs, start=False, stop=(tap == 24))
                        sl = slice(y0 * wo, y0 * wo + n)
                        g = {}
                        for ct in cts:
                            gt = gpool.tile([128, 480], F32, tag=f"g{ct}")
                            g[ct] = gt
                            if ct == 2:
                                nc.scalar.activation(gt[:, :n], ps[ct][:, :n], AF.Tanh,
                                                     bias=bsb[:, 2:3])
                            else:
                                nc.scalar.activation(gt[:, :n], ps[ct][:, :n], AF.Identity,
                                                     bias=bsig[:, ct:ct + 1], scale=0.2)
                                nc.vector.tensor_scalar(gt[:, :n], gt[:, :n], 0.0, 1.0,
                                                        ALU.max, ALU.min)
                        if t == 0:
                            nc.vector.tensor_mul(cst[b][:, sl], g[0][:, :n], g[2][:, :n])
                        else:
                            t1 = gpool.tile([128, 480], F32, tag="t1")
                            t2 = gpool.tile([128, 480], F32, tag="t2")
                            nc.vector.tensor_mul(t1[:, :n], g[1][:, :n], cst[b][:, sl])
                            nc.vector.tensor_mul(t2[:, :n], g[0][:, :n], g[2][:, :n])
                            nc.vector.tensor_add(cst[b][:, sl], t1[:, :n], t2[:, :n])
                        tc_t = gpool.tile([128, 480], F32, tag="tct")
                        nc.scalar.activation(tc_t[:, :n], cst[b][:, sl], AF.Tanh)
                        nc.vector.tensor_mul(hcur[b][:, sl], g[3][:, :n], tc_t[:, :n])
                    # end blocks: update padded state + spill sequence
                    dst = _tap_view(hpad[b], 2 * wp + 2, wo, wp, wo)
                    src = hcur[b][:, :].rearrange("p (r w) -> p r w", r=wo)
                    nc.vector.tensor_copy(dst, src)
                    nc.sync.dma_start(out=h1seq[b * T + t, :, :], in_=hcur[b][:, :])

        # ---------------- phase 2: ConvLSTM(128 -> 64), 56x56 ----------------
        with ExitStack() as ctx:
            wpool = ctx.enter_context(tc.tile_pool(name="w2", bufs=1))
            spool = ctx.enter_context(tc.tile_pool(name="s2", bufs=1))
            ipool = ctx.enter_context(tc.tile_pool(name="i2", bufs=2))
            gpool = ctx.enter_context(tc.tile_pool(name="g2", bufs=2))
            ppool = ctx.enter_context(tc.tile_pool(name="p2", bufs=8, space="PSUM"))

            wx2_t = wpool.tile([128, 25 * 256], F32R, tag="wx2")
            wh2_t = wpool.tile([128, 13 * 256], F32R, tag="wh2")
            nc.gpsimd.dma_start(out=wx2_t[:, :], in_=wx2[:, :])
            nc.gpsimd.dma_start(out=wh2_t[:, :], in_=wh2p[:, :])
            bsb = wpool.tile([64, 4], F32, tag="b2raw")
            bsig = wpool.tile([64, 4], F32, tag="b2sig")
            nc.sync.dma_start(out=bsb[:, :], in_=b2v[:, :])
            nc.vector.tensor_scalar(bsig[:, :], bsb[:, :], 0.2, 0.5, ALU.mult, ALU.add)

            hpad = [spool.tile([128, 3664], F32R, tag=f"hpad{b}", name=f"hpad2_{b}") for b in range(BC)]
            hpadB = [spool.tile([128, 3664], F32R, tag=f"hpadB{b}", name=f"hpad2B_{b}") for b in range(BC)]
            hcur = [spool.tile([64, 3136], F32R, tag=f"hcur{b}", name=f"hcur2_{b}") for b in range(BC)]
            cst = [spool.tile([64, 3136], F32, tag=f"c2_{b}", name=f"c2_{b}") for b in range(BC)]
            for b in range(BC):
                nc.gpsimd.memset(hpad[b][:, :].bitcast(F32), 0.0)
                nc.gpsimd.memset(hpadB[b][:, :].bitcast(F32), 0.0)

            wp, wo = L2["Wp"], L2["Wo"]
            for t in range(T):
                for b in range(BC):
                    img = ipool.tile([128, 3640], F32R, tag="h1in")
                    nc.sync.dma_start(out=img[:, :3600], in_=h1seq[b * T + t, :, :])
                    for y0, nrow in _blocks(56, 8):
                        n = nrow * wo
                        ps = []
                        for ct in range(2):
                            acc = ppool.tile([128, 448], F32, tag="ps")
                            ps.append(acc)
                            first = True
                            for tap in range(25):
                                dy, dx = divmod(tap, 5)
                                rhs = _tap_view(img, (y0 + dy) * 60 + dx, nrow, 60, wo)
                                nc.tensor.matmul(
                                    acc[:, :n],
                                    wx2_t[:, tap * 256 + ct * 128:tap * 256 + (ct + 1) * 128],
                                    rhs, start=first,
                                    stop=(t == 0 and tap == 24))
                                first = False
                            if t > 0:
                                for e in range(13):
                                    if e < 10:
                                        dy, k = divmod(e, 2)
                                        src, off = hpad[b], (y0 + dy) * wp + 2 * k
                                    elif e < 12:
                                        dy = 2 * (e - 10)
                                        src, off = hpadB[b], (y0 + dy) * wp + 4
                                    else:
                                        src, off = hpad[b], (y0 + 4) * wp + 4
                                    rhs = _tap_view(src, off, nrow, wp, wo)
                                    nc.tensor.matmul(
                                        acc[:, :n],
                                        wh2_t[:, e * 256 + ct * 128:e * 256 + (ct + 1) * 128],
                                        rhs, start=False, stop=(e == 12))
                        sl = slice(y0 * wo, y0 * wo + n)
                        # gates: ps[0]=[i;f], ps[1]=[c;o]
                        si = gpool.tile([64, 448], F32, tag="si")
                        nc.scalar.activation(si[:, :n], ps[0][0:64, :n], AF.Identity,
                                             bias=bsig[:, 0:1], scale=0.2)
                        nc.vector.tensor_scalar(si[:, :n], si[:, :n], 0.0, 1.0,
                                                ALU.max, ALU.min)
                        gt = gpool.tile([64, 448], F32, tag="gt")
                        nc.scalar.activation(gt[:, :n], ps[1][0:64, :n], AF.Tanh,
                                             bias=bsb[:, 2:3])
                        so = gpool.tile([64, 448], F32, tag="so")
                        nc.scalar.activation(so[:, :n], ps[1][64:128, :n], AF.Identity,
                                             bias=bsig[:, 3:4], scale=0.2)
                        nc.vector.tensor_scalar(so[:, :n], so[:, :n], 0.0, 1.0,
                                                ALU.max, ALU.min)
                        if t == 0:
                            nc.vector.tensor_mul(cst[b][:, sl], si[:, :n], gt[:, :n])
                        else:
                            sf = gpool.tile([64, 448], F32, tag="sf")
                            nc.scalar.activation(sf[:, :n], ps[0][64:128, :n], AF.Identity,
                                                 bias=bsig[:, 1:2], scale=0.2)
                            nc.vector.tensor_scalar(sf[:, :n], sf[:, :n], 0.0, 1.0,
                                                    ALU.max, ALU.min)
                            t1 = gpool.tile([64, 448], F32, tag="t1")
                            t2 = gpool.tile([64, 448], F32, tag="t2")
                            nc.vector.tensor_mul(t1[:, :n], sf[:, :n], cst[b][:, sl])
                            nc.vector.tensor_mul(t2[:, :n], si[:, :n], gt[:, :n])
                            nc.vector.tensor_add(cst[b][:, sl], t1[:, :n], t2[:, :n])
                        tc_t = gpool.tile([64, 448], F32, tag="tct")
                        nc.scalar.activation(tc_t[:, :n], cst[b][:, sl], AF.Tanh)
                        nc.vector.tensor_mul(hcur[b][:, sl], so[:, :n], tc_t[:, :n])
                    # end blocks: dup states (A: plain/x+1-shift, B: plain/row-shift)
                    src = hcur[b][:, :].rearrange("p (r w) -> p r w", r=wo)
                    dst0 = hpad[b][0:64, 2 * wp + 2:2 * wp + 2 + wo * wp] \
                        .rearrange("p (r w) -> p r w", r=wo)[:, :, :wo]
                    nc.vector.tensor_copy(dst0, src)
                    dst1 = hpad[b][64:128, 2 * wp + 1:2 * wp + 1 + wo * wp] \
                        .rearrange("p (r w) -> p r w", r=wo)[:, :, :wo]
                    nc.vector.tensor_copy(dst1, src)
                    dstB0 = hpadB[b][0:64, 2 * wp + 2:2 * wp + 2 + wo * wp] \
                        .rearrange("p (r w) -> p r w", r=wo)[:, :, :wo]
                    nc.vector.tensor_copy(dstB0, src)
                    dstB1 = hpadB[b][64:128, wp + 2:wp + 2 + wo * wp] \
                        .rearrange("p (r w) -> p r w", r=wo)[:, :, :wo]
                    nc.vector.tensor_copy(dstB1, src)
                    nc.sync.dma_start(out=h2seq[b * T + t, :, :], in_=hcur[b][:, :])

        # ---------------- phase 3: ConvLSTM(64 -> 64), 52x52 ----------------
        with ExitStack() as ctx:
            wpool = ctx.enter_context(tc.tile_pool(name="w3", bufs=1))
            spool = ctx.enter_context(tc.tile_pool(name="s3", bufs=1))
            ipool = ctx.enter_context(tc.tile_pool(name="i3", bufs=2))
            gpool = ctx.enter_context(tc.tile_pool(name="g3", bufs=2))
            ppool = ctx.enter_context(tc.tile_pool(name="p3", bufs=8, space="PSUM"))

            wx3_t = wpool.tile([128, 13 * 256], F32R, tag="wx3")
            wh3_t = wpool.tile([128, 13 * 256], F32R, tag="wh3")
            nc.gpsimd.dma_start(out=wx3_t[:, :], in_=wx3p[:, :])
            nc.gpsimd.dma_start(out=wh3_t[:, :], in_=wh3p[:, :])
            bsb = wpool.tile([64, 4], F32, tag="b3raw")
            bsig = wpool.tile([64, 4], F32, tag="b3sig")
            nc.sync.dma_start(out=bsb[:, :], in_=b3v[:, :])
            nc.vector.tensor_scalar(bsig[:, :], bsb[:, :], 0.2, 0.5, ALU.mult, ALU.add)

            hpad = [spool.tile([128, 3300], F32R, tag=f"hpad{b}", name=f"hpad3_{b}") for b in range(BC)]
            hpadB = [spool.tile([128, 3300], F32R, tag=f"hpadB{b}", name=f"hpad3B_{b}") for b in range(BC)]
            hcur = [spool.tile([64, 2704], F32R, tag=f"hcur{b}", name=f"hcur3_{b}") for b in range(BC)]
            cst = [spool.tile([64, 2704], F32, tag=f"c3_{b}", name=f"c3_{b}") for b in range(BC)]
            for b in range(BC):
                nc.gpsimd.memset(hpad[b][:, :].bitcast(F32), 0.0)
                nc.gpsimd.memset(hpadB[b][:, :].bitcast(F32), 0.0)

            wp, wo = L3["Wp"], L3["Wo"]
            for t in range(T):
                for b in range(BC):
                    # dup inputs [128, 3196]: img rows64 = x+1 shift, imgB rows64 =
                    # one-image-row shift; all four halves DMAed from DRAM.
                    img = ipool.tile([128, 3196], F32R, tag="h2in")
                    imgB = ipool.tile([128, 3196], F32R, tag="h2inB")
                    nc.sync.dma_start(out=img[0:64, :3136], in_=h2seq[b * T + t, :, :])
                    nc.sync.dma_start(out=img[64:128, :3135], in_=h2seq[b * T + t, :, 1:3136])
                    nc.sync.dma_start(out=imgB[0:64, :3136], in_=h2seq[b * T + t, :, :])
                    nc.sync.dma_start(out=imgB[64:128, :3080], in_=h2seq[b * T + t, :, 56:3136])
                    for y0, nrow in _blocks(52, 9):
                        n = nrow * wo
                        ps = []
                        for ct in range(2):
                            acc = ppool.tile([128, 468], F32, tag="ps")
                            ps.append(acc)
                            first = True
                            for e in range(13):
                                if e < 10:
                                    dy, k = divmod(e, 2)
                                    src, off = img, (y0 + dy) * 56 + 2 * k
                                elif e < 12:
                                    dy = 2 * (e - 10)
                                    src, off = imgB, (y0 + dy) * 56 + 4
                                else:
                                    src, off = img, (y0 + 4) * 56 + 4
                                stop13 = (t == 0 and e == 12)
                                if e == 12:
                                    # singleton tap (4,4): K=64 (upper half unwritten)
                                    rhs = src[0:64, off:off + nrow * 56] \
                                        .rearrange("p (r w) -> p r w", r=nrow)[:, :, :wo]
                                    nc.tensor.matmul(
                                        acc[:, :n],
                                        wx3_t[0:64, e * 256 + ct * 128:e * 256 + (ct + 1) * 128],
                                        rhs, start=first, stop=stop13)
                                else:
                                    rhs = _tap_view(src, off, nrow, 56, wo)
                                    nc.tensor.matmul(
                                        acc[:, :n],
                                        wx3_t[:, e * 256 + ct * 128:e * 256 + (ct + 1) * 128],
                                        rhs, start=first, stop=stop13)
                                first = False
                            if t > 0:
                                for e in range(13):
                                    if e < 10:
                                        dy, k = divmod(e, 2)
                                        src, off = hpad[b], (y0 + dy) * wp + 2 * k
                                    elif e < 12:
                                        dy = 2 * (e - 10)
                                        src, off = hpadB[b], (y0 + dy) * wp + 4
                                    else:
                                        src, off = hpad[b], (y0 + 4) * wp + 4
                                    rhs = _tap_view(src, off, nrow, wp, wo)
                                    nc.tensor.matmul(
                                        acc[:, :n],
                                        wh3_t[:, e * 256 + ct * 128:e * 256 + (ct + 1) * 128],
                                        rhs, start=False, stop=(e == 12))
                        sl = slice(y0 * wo, y0 * wo + n)
                        si = gpool.tile([64, 468], F32, tag="si")
                        nc.scalar.activation(si[:, :n], ps[0][0:64, :n], AF.Identity,
                                             bias=bsig[:, 0:1], scale=0.2)
                        nc.vector.tensor_scalar(si[:, :n], si[:, :n], 0.0, 1.0,
                                                ALU.max, ALU.min)
                        gt = gpool.tile([64, 468], F32, tag="gt")
                        nc.scalar.activation(gt[:, :n], ps[1][0:64, :n], AF.Tanh,
                                             bias=bsb[:, 2:3])
                        so = gpool.tile([64, 468], F32, tag="so")
                        nc.scalar.activation(so[:, :n], ps[1][64:128, :n], AF.Identity,
                                             bias=bsig[:, 3:4], scale=0.2)
                        nc.vector.tensor_scalar(so[:, :n], so[:, :n], 0.0, 1.0,
                                                ALU.max, ALU.min)
                        if t == 0:
                            nc.vector.tensor_mul(cst[b][:, sl], si[:, :n], gt[:, :n])
                        else:
                            sf = gpool.tile([64, 468], F32, tag="sf")
                            nc.scalar.activation(sf[:, :n], ps[0][64:128, :n], AF.Identity,
                                                 bias=bsig[:, 1:2], scale=0.2)
                            nc.vector.tensor_scalar(sf[:, :n], sf[:, :n], 0.0, 1.0,
                                                    ALU.max, ALU.min)
                            t1 = gpool.tile([64, 468], F32, tag="t1")
                            t2 = gpool.tile([64, 468], F32, tag="t2")
                            nc.vector.tensor_mul(t1[:, :n], sf[:, :n], cst[b][:, sl])
                            nc.vector.tensor_mul(t2[:, :n], si[:, :n], gt[:, :n])
                            nc.vector.tensor_add(cst[b][:, sl], t1[:, :n], t2[:, :n])
                        tc_t = gpool.tile([64, 468], F32, tag="tct")
                        nc.scalar.activation(tc_t[:, :n], cst[b][:, sl], AF.Tanh)
                        nc.vector.tensor_mul(hcur[b][:, sl], so[:, :n], tc_t[:, :n])
                    if t < T - 1:
                        src = hcur[b][:, :].rearrange("p (r w) -> p r w", r=wo)
                        dst0 = hpad[b][0:64, 2 * wp + 2:2 * wp + 2 + wo * wp] \
                            .rearrange("p (r w) -> p r w", r=wo)[:, :, :wo]
                        nc.vector.tensor_copy(dst0, src)
                        dst1 = hpad[b][64:128, 2 * wp + 1:2 * wp + 1 + wo * wp] \
                            .rearrange("p (r w) -> p r w", r=wo)[:, :, :wo]
                        nc.vector.tensor_copy(dst1, src)
                        dstB0 = hpadB[b][0:64, 2 * wp + 2:2 * wp + 2 + wo * wp] \
                            .rearrange("p (r w) -> p r w", r=wo)[:, :, :wo]
                        nc.vector.tensor_copy(dstB0, src)
                        dstB1 = hpadB[b][64:128, wp + 2:wp + 2 + wo * wp] \
                            .rearrange("p (r w) -> p r w", r=wo)[:, :, :wo]
                        nc.vector.tensor_copy(dstB1, src)
                    else:
                        nc.sync.dma_start(out=h3o[b * 64:(b + 1) * 64, :], in_=hcur[b][:, :])

    nc.compile()
    return nc


def TileCtx(nc):
    return tile.TileContext(nc, pool_alloc_mode="queue")


def _build_launch_b():
    """Column-sharded dense1 in bf16: core j computes a1[:, j*128:(j+1)*128].
    No collective; bias/relu/dense2/dense3 finish on host."""
    D = 173056
    KT = D // 128              # 1352 k-tiles
    CH = 64                    # k-tiles per weight DMA chunk
    NB = 4                     # interleaved PSUM accumulation banks
    nc = bacc.Bacc("TRN2", target_bir_lowering=False, debug=False,
                   num_devices=NCORES)
    ztk = nc.dram_tensor("ztk", [128, KT * 16], BF16, kind="ExternalInput").ap()
    wd1c = nc.dram_tensor("wd1c", [128, KT * 128], BF16, kind="ExternalInput").ap()
    out = nc.dram_tensor("out", [16, 128], F32, kind="ExternalOutput").ap()

    with TileCtx(nc) as tc, ExitStack() as ctx:
        cpool = ctx.enter_context(tc.tile_pool(name="cst", bufs=1))
        wpool = ctx.enter_context(tc.tile_pool(name="wd1", bufs=3))
        ppool = ctx.enter_context(tc.tile_pool(name="ps", bufs=1, space="PSUM"))

        zt = cpool.tile([128, KT * 16], BF16, tag="zt")
        nc.gpsimd.dma_start(out=zt[:, :], in_=ztk[:, :])
        accs = [ppool.tile([16, 128], F32, tag=f"a{b}", name=f"acc{b}")
                for b in range(NB)]

        nchunk = (KT + CH - 1) // CH
        for c in range(nchunk):
            c0 = c * CH
            cn = min(CH, KT - c0)
            w_t = wpool.tile([128, CH * 128], BF16, tag="w", name=f"w{c0}")
            nc.sync.dma_start(out=w_t[:, :cn * 128],
                              in_=wd1c[:, c0 * 128:(c0 + cn) * 128])
            for i in range(cn):
                kt = c0 + i
                nc.tensor.matmul(accs[kt % NB][:, :], zt[:, kt * 16:(kt + 1) * 16],
                                 w_t[:, i * 128:(i + 1) * 128],
                                 start=(kt < NB), stop=(kt >= KT - NB))
        sums = [cpool.tile([16, 128], F32, tag=f"s{i}", name=f"sum{i}")
                for i in range(NB)]
        nc.vector.tensor_copy(sums[0][:, :], accs[0][:, :])
        for i in range(1, NB):
            nc.vector.tensor_add(sums[i][:, :], sums[i - 1][:, :], accs[i][:, :])
        nc.sync.dma_start(out=out[:, :], in_=sums[NB - 1][:, :])

    nc.compile()
    return nc


def _pack13(w):
    """(5,5,64,256) -> [128, 13*256].
    e<10: in-row pairs (dy,2k)+(dy,2k+1) via the shift-1 dup (k=divmod).
    e=10,11: cross-row pairs (dy,4)+(dy+1,4) via the shift-wp dup (dy=0,2).
    e=12: singleton (4,4), upper half zero."""
    out = np.zeros((128, 13, 256), np.float32)
    for e in range(10):
        dy, k = divmod(e, 2)
        out[0:64, e] = w[dy, 2 * k]
        out[64:128, e] = w[dy, 2 * k + 1]
    for i, dy in enumerate((0, 2)):
        out[0:64, 10 + i] = w[dy, 4]
        out[64:128, 10 + i] = w[dy + 1, 4]
    out[0:64, 12] = w[4, 4]
    return np.ascontiguousarray(out.reshape(128, 13 * 256))


def _host_prep_a(x, Wx1, Wh1, b1, Wx2, Wh2, b2, Wx3, Wh3, b3):
    xw = np.lib.stride_tricks.sliding_window_view(x, (5, 5), axis=(2, 3))
    # [b,t,y,x,c,dy,dx] -> [b,t,(dy,dx,c),(y,x)]
    xim = np.ascontiguousarray(
        xw.transpose(0, 1, 5, 6, 4, 2, 3).reshape(16, 6, 125, 3600), np.float32)
    shared = dict(
        wx1=np.ascontiguousarray(Wx1.reshape(125, 512), np.float32),
        wh1=np.ascontiguousarray(
            Wh1.reshape(25, 128, 512).transpose(1, 0, 2).reshape(128, 25 * 512)),
        wx2=np.ascontiguousarray(
            Wx2.reshape(25, 128, 256).transpose(1, 0, 2).reshape(128, 25 * 256)),
        wh2p=_pack13(Wh2.reshape(5, 5, 64, 256)),
        wx3p=_pack13(Wx3.reshape(5, 5, 64, 256)),
        wh3p=_pack13(Wh3.reshape(5, 5, 64, 256)),
        b1v=np.ascontiguousarray(b1.reshape(4, 128).T, np.float32),
        b2v=np.ascontiguousarray(b2.reshape(4, 64).T, np.float32),
        b3v=np.ascontiguousarray(b3.reshape(4, 64).T, np.float32),
    )
    in_maps = []
    for j in range(NCORES):
        m = dict(shared)
        m["xim"] = np.ascontiguousarray(
            xim[2 * j:2 * j + 2].reshape(12, 125, 3600))
        in_maps.append(m)
    return in_maps


def _run(nc, in_maps, trace):
    res = run_bass_kernel_spmd(nc, in_maps, core_ids=list(range(NCORES)),
                               trace=trace)
    if res.exec_time_ns is not None:
        LAST_EXEC_NS.append(res.exec_time_ns)
    LAST_RESULTS.append(res)
    return res


def kernel(x, Wx1, Wh1, b1, Wx2, Wh2, b2, Wx3, Wh3, b3,
           Wd1, bd1, Wd2, bd2, Wd3, bd3):
    trace = _want_trace()
    LAST_EXEC_NS.clear()
    LAST_RESULTS.clear()
    x = np.asarray(x, np.float32)

    if "a" not in _CACHE:
        _CACHE["a"] = _build_launch_a()
    in_a = _host_prep_a(x, np.asarray(Wx1), np.asarray(Wh1), np.asarray(b1),
                        np.asarray(Wx2), np.asarray(Wh2), np.asarray(b2),
                        np.asarray(Wx3), np.asarray(Wh3), np.asarray(b3))
    res_a = _run(_CACHE["a"], in_a, trace)

    h3 = np.stack([res_a.results[j]["h3o"][(b % 2) * 64:(b % 2) * 64 + 64]
                   for b, j in [(b, b // 2) for b in range(16)]])  # [16,64,2704]
    zt = np.ascontiguousarray(h3.transpose(2, 1, 0).reshape(173056, 16), np.float32)

    if "b" not in _CACHE:
        _CACHE["b"] = _build_launch_b()
    D = 173056
    KT = D // 128
    w_b = np.asarray(Wd1, np.float32).astype(BFNP)
    ztk = np.ascontiguousarray(
        zt.astype(BFNP).reshape(KT, 128, 16).transpose(1, 0, 2).reshape(128, KT * 16))
    in_b = []
    for j in range(NCORES):
        shard = w_b[:, j * 128:(j + 1) * 128]
        in_b.append(dict(ztk=ztk, wd1c=np.ascontiguousarray(
            shard.reshape(KT, 128, 128).transpose(1, 0, 2).reshape(128, KT * 128))))
    res_b = _run(_CACHE["b"], in_b, trace)

    a1 = np.concatenate([res_b.results[j]["out"] for j in range(NCORES)], axis=1)
    a1 = np.maximum(a1 + np.asarray(bd1, np.float32), 0.0)
    a2 = np.maximum(a1 @ np.asarray(Wd2, np.float32) + np.asarray(bd2, np.float32), 0.0)
    out = a2 @ np.asarray(Wd3, np.float32) + np.asarray(bd3, np.float32)
    return np.ascontiguousarray(out, np.float32)

